# revision 4
# baseline (speedup 1.0000x reference)
"""CAM (channel attention) module kernel for Trainium2, 8 NeuronCores.

Reference computation (per sample, x: [C, N] with C=512, N=64*64):
    energy    = x @ x.T                      # [C, C] symmetric Gram matrix
    energy_n  = rowmax(energy) - energy
    att       = softmax(energy_n, axis=-1)
    out       = gamma * (att @ x) + x

Softmax shift-invariance: softmax(rowmax - e) == softmax(-e), stabilized
with the row-min m_i:  att[i,j] = exp(m_i - e_ij) / S_i,  S_i = sum_j.

Sharding: pure data parallel over batch B=16 -> 2 samples per core.

Per-core pipeline (matmul path in bf16, energy/softmax/epilogue in f32):
  1. x streams in as [128, 512] f32 tiles in q-major order; the Scalar
     engine converts each tile to bf16 (natbf) as it lands.  natbf later
     doubles as mm2's rhs, so no separate re-round pass exists.
  2. per 128-col k-chunk: 4 PE transposes (bf16, 1 cyc/row) -> PSUM,
     one DVE copy -> xt (bf16 SBUF, 2x_1p fast path), then ALL FOUR
     triangular Gram panels accumulate their k-th term (free dims
     512/384/256/128 - bf16 pays no penalty below 256).  Interleaving
     the panels into the k-loop matches PE pace to the input-DMA pace,
     so even the cold first sample never starves the PE.
  3. energy is symmetric: panel ci computes cols [128*ci:512]; the
     missing lower blocks are mirrored from finished panels via f32 PE
     transposes of SBUF-staged blocks.
  4. softmax per panel: m = rowmin (DVE); P = exp(m - e) with fused
     row-sum S (ACT, reads PSUM, writes bf16); D = identity*(gamma/S).
  5. PT = P.T @ D on the PE (folds softmax normalization AND gamma into
     the transpose); PSUM -> bf16 SBUF copies on ACT.
  6. mm2: out_tile = sum_bj PT[bj][:,ci].T @ natbf[nt,bj]; epilogue
     out = psum + x on DVE (GPSIMD cannot read PSUM); the output DMAs
     dispatch from the otherwise-idle Pool queue (cheap sequencer).
     nt-outer order frees nat tiles in the exact order the next
     sample's input DMA wants them, so the next load streams behind
     the epilogue wavefront.
  x stays exact fp32 end-to-end into the epilogue, so gamma=0
  reproduces x bit-exactly.

Note on precision: bf16 matmul inputs with f32 PSUM accumulation give
~1e-2 worst-case relative error on the attention path for a nonzero
gamma (the Gram energies see ~0.1 absolute error through the exp);
with the module's gamma=0 the output equals x exactly.  Full-f32
matmuls would be 4x slower on the PE, f32r ~equal for the big panels
but 2-4x slower for transposes and sub-256 tiles.
"""

import numpy as np

import concourse.bacc as bacc
import concourse.tile as tile
from concourse import mybir
from concourse.bass_utils import run_bass_kernel_spmd
from concourse.masks import make_identity

B, C, H, W = 16, 512, 64, 64
N = H * W
NCORES = 8
BPC = B // NCORES  # samples per core
CB = C // 128      # channel blocks (4)
NK = N // 128      # 128-wide n-chunks (32)
NT = N // 512      # 512-wide n-tiles (8)

F32 = mybir.dt.float32
BF16 = mybir.dt.bfloat16


def _emit(nc, tc, ctx, x, gamma, out):
    consts = ctx.enter_context(tc.tile_pool(name="consts", bufs=1))
    nat_pool = ctx.enter_context(tc.tile_pool(name="nat", bufs=40))
    nbf_pool = ctx.enter_context(tc.tile_pool(name="nbf", bufs=36))
    xt_pool = ctx.enter_context(tc.tile_pool(name="xt", bufs=6))
    p_pool = ctx.enter_context(tc.tile_pool(name="p", bufs=5))
    eblk_pool = ctx.enter_context(tc.tile_pool(name="eblk", bufs=7))
    pt_pool = ctx.enter_context(tc.tile_pool(name="pt", bufs=5))
    d_pool = ctx.enter_context(tc.tile_pool(name="d", bufs=5))
    small = ctx.enter_context(tc.tile_pool(name="small", bufs=4 * CB + 2))
    outs_pool = ctx.enter_context(tc.tile_pool(name="outs", bufs=3))
    psum_e = ctx.enter_context(tc.tile_pool(name="psum_e", bufs=4, space="PSUM"))
    psum_t = ctx.enter_context(tc.tile_pool(name="psum_t", bufs=2, space="PSUM"))
    psum_g = ctx.enter_context(tc.tile_pool(name="psum_g", bufs=2, space="PSUM"))

    identity = consts.tile([128, 128], F32)
    make_identity(nc, identity[:])
    idbf = consts.tile([128, 128], BF16)
    nc.vector.tensor_copy(out=idbf[:], in_=identity[:])
    g_sb = consts.tile([128, 1], F32)
    nc.gpsimd.dma_start(out=g_sb[:], in_=gamma[:].to_broadcast((128, 1)))

    for s in range(BPC):
        # ---- input stream: [128,512] f32 tiles q-major; bf16 copies on ACT
        nat = {}
        nbf = {}
        for q in range(NT):
            for c in range(CB):
                t = nat_pool.tile([128, 512], F32, tag="nat", name=f"nat{s}_{q}_{c}")
                nc.sync.dma_start(
                    out=t[:],
                    in_=x[s, 128 * c : 128 * (c + 1), 512 * q : 512 * (q + 1)],
                )
                nat[(q, c)] = t
                b = nbf_pool.tile([128, 512], BF16, tag="nbf", name=f"nbf{s}_{q}_{c}")
                nc.scalar.activation(
                    out=b[:], in_=t[:],
                    func=mybir.ActivationFunctionType.Copy,
                    bias=0.0, scale=1.0,
                )
                nbf[(q, c)] = b

        # keep the PE clock ramped across the sample boundary
        warm_ps = psum_g.tile([128, 128], F32, tag="g", name=f"warm{s}")
        nwarm = 16 if s == 0 else 6
        for w in range(nwarm):
            nc.tensor.matmul(warm_ps[:], idbf[:], idbf[:], start=(w == 0), stop=False)
        nc.tensor.matmul(warm_ps[:], idbf[:], idbf[:], start=False, stop=True)

        # ---- per k-chunk: 4 bf16 transposes + one xt copy + all 4
        # triangular Gram panels' k-th accumulation term ----
        e_ps = [
            psum_e.tile([128, C], F32, tag="e", name=f"e{s}_{ci}")
            for ci in range(CB)
        ]
        for k in range(NK):
            q, r = divmod(k, 4)
            t_ps = psum_t.tile([128, C], BF16, tag="t")
            for c in range(CB):
                nc.tensor.transpose(
                    t_ps[:, 128 * c : 128 * (c + 1)],
                    nbf[(q, c)][:, 128 * r : 128 * (r + 1)],
                    idbf[:],
                )
            xt = xt_pool.tile([128, C], BF16, tag="xt")
            nc.vector.tensor_copy(out=xt[:], in_=t_ps[:])
            for ci in range(CB):
                lo = 128 * ci
                nc.tensor.matmul(
                    e_ps[ci][:, lo:C],
                    xt[:, lo : lo + 128],
                    xt[:, lo:C],
                    start=(k == 0),
                    stop=(k == NK - 1),
                )

        # ---- mirror the lower-triangle blocks from finished panels ----
        e_blk = {}
        for cj in range(CB):
            for ci in range(cj + 1, CB):
                blk = eblk_pool.tile(
                    [128, 128], F32, tag="eblk", name=f"eblk{s}_{cj}_{ci}"
                )
                nc.vector.tensor_copy(
                    out=blk[:], in_=e_ps[cj][:, 128 * ci : 128 * (ci + 1)]
                )
                e_blk[(cj, ci)] = blk
        for ci in range(1, CB):
            for cj in range(ci):
                nc.tensor.transpose(
                    e_ps[ci][:, 128 * cj : 128 * (cj + 1)],
                    e_blk[(cj, ci)][:],
                    identity[:],
                )

        # ---- softmax pieces: P = exp(m - e) bf16, S = rowsum,
        #      D = identity * (gamma/S) ----
        p_t = []
        d_t = []
        for ci in range(CB):
            m = small.tile([128, 1], F32, tag="m")
            nc.vector.tensor_reduce(
                out=m[:], in_=e_ps[ci][:], axis=mybir.AxisListType.X,
                op=mybir.AluOpType.min,
            )
            p = p_pool.tile([128, C], BF16, tag="p")
            ssum = small.tile([128, 1], F32, tag="s")
            nc.scalar.activation(
                out=p[:], in_=e_ps[ci][:],
                func=mybir.ActivationFunctionType.Exp,
                bias=m[:], scale=-1.0, accum_out=ssum[:],
            )
            rcp = small.tile([128, 1], F32, tag="r")
            nc.vector.reciprocal(out=rcp[:], in_=ssum[:])
            gv = small.tile([128, 1], F32, tag="gv")
            nc.vector.tensor_mul(out=gv[:], in0=rcp[:], in1=g_sb[:])
            d = d_pool.tile([128, 128], BF16, tag="d")
            nc.vector.tensor_scalar_mul(out=d[:], in0=identity[:], scalar1=gv[:])
            p_t.append(p)
            d_t.append(d)

        # ---- PT = P.T @ diag(gamma/S): PT[j, i] = gamma * att[i, j] ----
        ptps = [
            psum_e.tile([128, C], F32, tag="e", name=f"ptp{s}_{bj}")
            for bj in range(CB)
        ]
        for bi in range(CB):
            for bj in range(CB):
                nc.tensor.matmul(
                    ptps[bj][:, 128 * bi : 128 * (bi + 1)],
                    p_t[bi][:, 128 * bj : 128 * (bj + 1)],
                    d_t[bi][:],
                    start=True,
                    stop=True,
                )
        pt = []
        for bj in range(CB):
            ptt = pt_pool.tile([128, C], BF16, tag="pt", name=f"ptt{s}_{bj}")
            nc.scalar.activation(
                out=ptt[:], in_=ptps[bj][:],
                func=mybir.ActivationFunctionType.Copy,
                bias=0.0, scale=1.0,
            )
            pt.append(ptt)

        # ---- out = PT.T @ natbf + x; epilogue + output DMA on Pool ----
        for nt in range(NT):
            for ci in range(CB):
                ops = psum_g.tile([128, 512], F32, tag="g")
                for bj in range(CB):
                    nc.tensor.matmul(
                        ops[:],
                        pt[bj][:, 128 * ci : 128 * (ci + 1)],
                        nbf[(nt, bj)][:],
                        start=(bj == 0),
                        stop=(bj == CB - 1),
                    )
                o_sb = outs_pool.tile([128, 512], F32, tag="o")
                nc.vector.scalar_tensor_tensor(
                    out=o_sb[:],
                    in0=ops[:],
                    scalar=1.0,
                    in1=nat[(nt, ci)][:],
                    op0=mybir.AluOpType.bypass,
                    op1=mybir.AluOpType.add,
                )
                nc.gpsimd.dma_start(
                    out=out[
                        s, 128 * ci : 128 * (ci + 1), 512 * nt : 512 * (nt + 1)
                    ],
                    in_=o_sb[:],
                )


_NC_CACHE = None


def _build():
    global _NC_CACHE
    if _NC_CACHE is not None:
        return _NC_CACHE
    from contextlib import ExitStack

    nc = bacc.Bacc("TRN2", target_bir_lowering=False)
    x = nc.dram_tensor("x", [BPC, C, N], F32, kind="ExternalInput")
    gamma = nc.dram_tensor("gamma", [1, 1], F32, kind="ExternalInput")
    out = nc.dram_tensor("out", [BPC, C, N], F32, kind="ExternalOutput")
    with tile.TileContext(nc) as tc:
        with ExitStack() as ctx:
            _emit(nc, tc, ctx, x[:], gamma[:], out[:])
    nc.compile()
    _NC_CACHE = nc
    return nc


def kernel(x, gamma):
    x = np.ascontiguousarray(np.asarray(x, dtype=np.float32))
    gamma = np.ascontiguousarray(np.asarray(gamma, dtype=np.float32))
    assert x.shape == (B, C, H, W), x.shape
    xf = x.reshape(B, C, N)
    nc = _build()
    in_maps = [
        {
            "x": xf[c * BPC : (c + 1) * BPC],
            "gamma": gamma.reshape(1, 1),
        }
        for c in range(NCORES)
    ]
    res = run_bass_kernel_spmd(nc, in_maps, core_ids=list(range(NCORES)))
    out = np.concatenate([res.results[c]["out"] for c in range(NCORES)], axis=0)
    return out.reshape(B, C, H, W)


# revision 8
# speedup vs baseline: 1.2087x; 1.2087x over previous
"""CAM (channel attention) module kernel for Trainium2, 8 NeuronCores.

Reference computation (per sample, x: [C, N] with C=512, N=64*64):
    energy    = x @ x.T                      # [C, C] symmetric Gram matrix
    energy_n  = rowmax(energy) - energy
    att       = softmax(energy_n, axis=-1)
    out       = gamma * (att @ x) + x

Softmax shift-invariance: softmax(rowmax - e) == softmax(-e), stabilized
with the row-min m_i:  att[i,j] = exp(m_i - e_ij) / S_i,  S_i = sum_j.

Sharding: pure data parallel over batch B=16 -> 2 samples per core.

Per-core pipeline (matmul path in bf16, energy/softmax/epilogue in f32):
  1. x streams in as [128, 2048] f32 q-tiles (4 channel blocks batched
     into ONE dma_start via a [128, 4, 512] DRAM access pattern - the
     sync sequencer costs ~650ns per dispatch, so per-[128,512] issues
     would rate-limit the stream); the Scalar engine converts each
     q-tile to bf16 (natbf) as it lands.  natbf doubles as mm2's rhs.
  2. per 128-col k-chunk: 4 PE transposes (bf16, 1 cyc/row) -> PSUM,
     one DVE copy -> xt (bf16 SBUF, 2x_1p fast path), then ALL FOUR
     triangular Gram panels accumulate their k-th term (free dims
     512/384/256/128 - bf16 pays no penalty below 256).  Interleaving
     panels into the k-loop matches PE pace to the input-DMA pace;
     psum_t bufs=3 gives the scheduler transpose lookahead to hide the
     xt-copy latency (a per-chunk PE gap would also knock the HAM
     clock off its 2.4 GHz pstate).
  3. energy is symmetric: panel ci computes cols [128*ci:512]; the
     missing lower blocks are mirrored from finished panels via f32 PE
     transposes of SBUF-staged blocks.
  4. softmax per panel: m = rowmin (DVE); P = exp(m - e) with fused
     row-sum S (ACT, reads PSUM, writes bf16); D = identity*(gamma/S).
     Dummy warm matmuls fill the PE quiet zone so the HAM pstate
     machine never sees an idle window here.
  5. PT = P.T @ D on the PE (folds softmax normalization AND gamma into
     the transpose); PSUM -> bf16 SBUF copies on ACT.
  6. mm2: out_tile = sum_bj PT[bj][:,ci].T @ natbf[nt,bj]; epilogue
     out = psum + x on DVE (GPSIMD cannot read PSUM) into [128,2048]
     nt-tiles; one batched output DMA per nt from the Pool queue
     (except the final tile, issued per-quarter to shorten the drain).
     nt-outer order frees nat q-tiles in the exact order the next
     sample's input DMA wants them, so the next load streams behind
     the epilogue wavefront.
  x stays exact fp32 end-to-end into the epilogue, so gamma=0
  reproduces x bit-exactly.

Note on precision: bf16 matmul inputs with f32 PSUM accumulation give
~1e-2 worst-case relative error on the attention path for a nonzero
gamma; with the module's gamma=0 the output equals x exactly.  Full-f32
matmuls would be 4x slower on the PE, f32r ~equal for the big panels
but 2-4x slower for transposes and sub-256 tiles.
"""

import numpy as np

import concourse.bacc as bacc
import concourse.tile as tile
from concourse import mybir
from concourse.bass_utils import run_bass_kernel_spmd
from concourse.masks import make_identity

B, C, H, W = 16, 512, 64, 64
N = H * W
NCORES = 8
BPC = B // NCORES  # samples per core
CB = C // 128      # channel blocks (4)
NK = N // 128      # 128-wide n-chunks (32)
NT = N // 512      # 512-wide n-tiles (8)

F32 = mybir.dt.float32
BF16 = mybir.dt.bfloat16


def _warm(nc, psum_pool, idbf, n, tag, name):
    """n dummy bf16 matmuls: keeps the PE HAM pstate alive through a
    window where real matmuls are blocked on other engines."""
    if n <= 0:
        return
    warm_ps = psum_pool.tile([128, 128], F32, tag=tag, name=name)
    for w in range(n):
        nc.tensor.matmul(warm_ps[:], idbf[:], idbf[:], start=(w == 0), stop=False)
    nc.tensor.matmul(warm_ps[:], idbf[:], idbf[:], start=False, stop=True)


def _emit(nc, tc, ctx, x, gamma, out):
    consts = ctx.enter_context(tc.tile_pool(name="consts", bufs=1))
    nat_pool = ctx.enter_context(tc.tile_pool(name="nat", bufs=10))
    nbf_pool = ctx.enter_context(tc.tile_pool(name="nbf", bufs=9))
    xt_pool = ctx.enter_context(tc.tile_pool(name="xt", bufs=4))
    p_pool = ctx.enter_context(tc.tile_pool(name="p", bufs=5))
    eblk_pool = ctx.enter_context(tc.tile_pool(name="eblk", bufs=7))
    pt_pool = ctx.enter_context(tc.tile_pool(name="pt", bufs=5))
    d_pool = ctx.enter_context(tc.tile_pool(name="d", bufs=5))
    small = ctx.enter_context(tc.tile_pool(name="small", bufs=4 * CB + 2))
    outs_pool = ctx.enter_context(tc.tile_pool(name="outs", bufs=3))
    psum_e = ctx.enter_context(tc.tile_pool(name="psum_e", bufs=4, space="PSUM"))
    psum_t = ctx.enter_context(tc.tile_pool(name="psum_t", bufs=2, space="PSUM"))
    psum_g = ctx.enter_context(tc.tile_pool(name="psum_g", bufs=2, space="PSUM"))

    identity = consts.tile([128, 128], F32)
    make_identity(nc, identity[:])
    idbf = consts.tile([128, 128], BF16)
    nc.vector.tensor_copy(out=idbf[:], in_=identity[:])
    g_sb = consts.tile([128, 1], F32)
    nc.gpsimd.dma_start(out=g_sb[:], in_=gamma[:].to_broadcast((128, 1)))

    for s in range(BPC):
        # ---- input stream: [128, 2048] f32 q-tiles (one dispatch each),
        # bf16 copies on ACT as they land ----
        nat = {}
        nbf = {}
        for q in range(NT):
            t = nat_pool.tile([128, 4 * 512], F32, tag="nat", name=f"nat{s}_{q}")
            src = x[s, :, :, 512 * q : 512 * (q + 1)].transpose([1, 0, 2])
            nc.sync.dma_start(out=t[:], in_=src)
            nat[q] = t
            b = nbf_pool.tile([128, 4 * 512], BF16, tag="nbf", name=f"nbf{s}_{q}")
            nc.scalar.activation(
                out=b[:], in_=t[:],
                func=mybir.ActivationFunctionType.Copy,
                bias=0.0, scale=1.0,
            )
            nbf[q] = b

        # keep the PE clock ramped across the sample boundary
        _warm(nc, psum_g, idbf, 16 if s == 0 else 6, "g", f"warm{s}")

        # ---- per k-chunk: 4 bf16 transposes + one xt copy + all 4
        # triangular Gram panels' k-th accumulation term ----
        e_ps = [
            psum_e.tile([128, C], F32, tag="e", name=f"e{s}_{ci}")
            for ci in range(CB)
        ]
        # two k-chunks share one [128,1024] PSUM transpose tile (= exactly
        # one PSUM bank) and one DVE copy; bufs=2 gives a 2-pair lookahead
        for j in range(NK // 2):
            t_ps = psum_t.tile([128, 2 * C], BF16, tag="t")
            for h in range(2):
                q, r = divmod(2 * j + h, 4)
                for c in range(CB):
                    nc.tensor.transpose(
                        t_ps[:, 512 * h + 128 * c : 512 * h + 128 * (c + 1)],
                        nbf[q][:, 512 * c + 128 * r : 512 * c + 128 * (r + 1)],
                        idbf[:],
                    )
            xt = xt_pool.tile([128, 2 * C], BF16, tag="xt")
            nc.vector.tensor_copy(out=xt[:], in_=t_ps[:])
            for h in range(2):
                k = 2 * j + h
                base = 512 * h
                for ci in range(CB):
                    lo = 128 * ci
                    nc.tensor.matmul(
                        e_ps[ci][:, lo:C],
                        xt[:, base + lo : base + lo + 128],
                        xt[:, base + lo : base + C],
                        start=(k == 0),
                        stop=(k == NK - 1),
                    )

        # ---- mirror the lower-triangle blocks from finished panels ----
        e_blk = {}
        for cj in range(CB):
            for ci in range(cj + 1, CB):
                blk = eblk_pool.tile(
                    [128, 128], F32, tag="eblk", name=f"eblk{s}_{cj}_{ci}"
                )
                nc.vector.tensor_copy(
                    out=blk[:], in_=e_ps[cj][:, 128 * ci : 128 * (ci + 1)]
                )
                e_blk[(cj, ci)] = blk
        for ci in range(1, CB):
            for cj in range(ci):
                nc.tensor.transpose(
                    e_ps[ci][:, 128 * cj : 128 * (cj + 1)],
                    e_blk[(cj, ci)][:],
                    identity[:],
                )

        # ---- softmax pieces: P = exp(m - e) bf16, S = rowsum,
        #      D = identity * (gamma/S); warm-fill the PE quiet zone ----
        _warm(nc, psum_g, idbf, 24, "g", f"warmS{s}")
        p_t = []
        d_t = []
        for ci in range(CB):
            m = small.tile([128, 1], F32, tag="m")
            nc.vector.tensor_reduce(
                out=m[:], in_=e_ps[ci][:], axis=mybir.AxisListType.X,
                op=mybir.AluOpType.min,
            )
            p = p_pool.tile([128, C], BF16, tag="p")
            ssum = small.tile([128, 1], F32, tag="s")
            nc.scalar.activation(
                out=p[:], in_=e_ps[ci][:],
                func=mybir.ActivationFunctionType.Exp,
                bias=m[:], scale=-1.0, accum_out=ssum[:],
            )
            rcp = small.tile([128, 1], F32, tag="r")
            nc.vector.reciprocal(out=rcp[:], in_=ssum[:])
            gv = small.tile([128, 1], F32, tag="gv")
            nc.vector.tensor_mul(out=gv[:], in0=rcp[:], in1=g_sb[:])
            d = d_pool.tile([128, 128], BF16, tag="d")
            nc.vector.tensor_scalar_mul(out=d[:], in0=identity[:], scalar1=gv[:])
            p_t.append(p)
            d_t.append(d)

        # ---- PT = P.T @ diag(gamma/S): PT[j, i] = gamma * att[i, j] ----
        ptps = [
            psum_e.tile([128, C], F32, tag="e", name=f"ptp{s}_{bj}")
            for bj in range(CB)
        ]
        for bi in range(CB):
            for bj in range(CB):
                nc.tensor.matmul(
                    ptps[bj][:, 128 * bi : 128 * (bi + 1)],
                    p_t[bi][:, 128 * bj : 128 * (bj + 1)],
                    d_t[bi][:],
                    start=True,
                    stop=True,
                )
        pt = []
        for bj in range(CB):
            ptt = pt_pool.tile([128, C], BF16, tag="pt", name=f"ptt{s}_{bj}")
            nc.scalar.activation(
                out=ptt[:], in_=ptps[bj][:],
                func=mybir.ActivationFunctionType.Copy,
                bias=0.0, scale=1.0,
            )
            pt.append(ptt)
        _warm(nc, psum_g, idbf, 8, "g", f"warmT{s}")

        # ---- out = PT.T @ natbf + x; epilogue on DVE; batched out DMA ----
        for nt in range(NT):
            o_sb = outs_pool.tile([128, 4 * 512], F32, tag="o", name=f"o{s}_{nt}")
            last = s == BPC - 1 and nt == NT - 1
            for ci in range(CB):
                ops = psum_g.tile([128, 512], F32, tag="g")
                for bj in range(CB):
                    nc.tensor.matmul(
                        ops[:],
                        pt[bj][:, 128 * ci : 128 * (ci + 1)],
                        nbf[nt][:, 512 * bj : 512 * (bj + 1)],
                        start=(bj == 0),
                        stop=(bj == CB - 1),
                    )
                nc.vector.scalar_tensor_tensor(
                    out=o_sb[:, 512 * ci : 512 * (ci + 1)],
                    in0=ops[:],
                    scalar=1.0,
                    in1=nat[nt][:, 512 * ci : 512 * (ci + 1)],
                    op0=mybir.AluOpType.bypass,
                    op1=mybir.AluOpType.add,
                )
                if last:
                    # final tile: per-quarter DMAs shorten the drain
                    nc.gpsimd.dma_start(
                        out=out[s, ci, :, 512 * nt : 512 * (nt + 1)],
                        in_=o_sb[:, 512 * ci : 512 * (ci + 1)],
                    )
            if not last:
                dst = out[s, :, :, 512 * nt : 512 * (nt + 1)].transpose([1, 0, 2])
                nc.gpsimd.dma_start(out=dst, in_=o_sb[:])


_NC_CACHE = None


def _build():
    global _NC_CACHE
    if _NC_CACHE is not None:
        return _NC_CACHE
    from contextlib import ExitStack

    nc = bacc.Bacc("TRN2", target_bir_lowering=False)
    x = nc.dram_tensor("x", [BPC, CB, 128, N], F32, kind="ExternalInput")
    gamma = nc.dram_tensor("gamma", [1, 1], F32, kind="ExternalInput")
    out = nc.dram_tensor("out", [BPC, CB, 128, N], F32, kind="ExternalOutput")
    with tile.TileContext(nc) as tc:
        with ExitStack() as ctx:
            _emit(nc, tc, ctx, x[:], gamma[:], out[:])
    nc.compile()
    _NC_CACHE = nc
    return nc


def kernel(x, gamma):
    x = np.ascontiguousarray(np.asarray(x, dtype=np.float32))
    gamma = np.ascontiguousarray(np.asarray(gamma, dtype=np.float32))
    assert x.shape == (B, C, H, W), x.shape
    xf = x.reshape(B, CB, 128, N)
    nc = _build()
    in_maps = [
        {
            "x": xf[c * BPC : (c + 1) * BPC],
            "gamma": gamma.reshape(1, 1),
        }
        for c in range(NCORES)
    ]
    res = run_bass_kernel_spmd(nc, in_maps, core_ids=list(range(NCORES)))
    out = np.concatenate([res.results[c]["out"] for c in range(NCORES)], axis=0)
    return out.reshape(B, C, H, W)
